# revision 37
# baseline (speedup 1.0000x reference)
"""Grouped-dequant GEMM (y = x @ (W * group_scales)^T + bias) on 8 TRN2 NeuronCores.

Tensor-parallel (column) sharding: each core owns O/8 = 512 output features.
x is replicated; weight/scales/bias are sharded along out_features; output
shards are concatenated on the host. All FLOPs (dequant multiply, GEMM, bias
add) run on device; the host only does sharding + layout transforms.

Self-contained: hardcodes shapes from the problem spec.
  x      (4, 2048, 4096) fp16
  weight (4096, 4096)    fp16
  scales (4096, 32)      fp16   group size g=128 along in_features
  bias   (4096,)         fp16
  types  (64, 32)        int32  (unused by the exact-dequant reference math)
"""

import sys
import types as _types

sys.path.insert(0, "/opt/trn_rl_repo")


def _install_ntff_hook_shim():
    """antenv.axon_hooks is missing in this image; register the NTFF profile
    hook from trn_agent_boot so run_bass_kernel_spmd(trace=True) works."""
    if "antenv.axon_hooks" in sys.modules:
        return
    mod = _types.ModuleType("antenv.axon_hooks")
    try:
        import trn_agent_boot.trn_boot as tb

        hook = tb._ntff_profile_via_ctypes("/opt/axon/libaxon_pjrt.so")
    except Exception:
        hook = None
    mod.get_axon_ntff_profile_hook = lambda: hook
    mod.set_axon_ntff_profile_hook = lambda h: None
    sys.modules["antenv.axon_hooks"] = mod


_install_ntff_hook_shim()

import numpy as np

import concourse.bacc as bacc
import concourse.mybir as mybir
import concourse.tile as tile
from concourse.bass import ds, ts
from concourse.bass_utils import run_bass_kernel_spmd
from concourse.bass import _add_dep_helper
from concourse.kernels.tile_matmul import (
    ShapeInfo,
    composable_matmul_tile_kernel,
    dma_from_dram_kxn,
)

B, S, I, O, G = 4, 2048, 4096, 4096, 128
N_CORES = 8
OC = O // N_CORES  # 512 output features per core
M = B * S  # 8192 tokens
P = 128

_cached_nc = None


def _build_bass():
    """Build + compile the per-core Bass program (same graph on all 8 cores).

    Computes y = xT.T @ w_deqT + bias where
      kxm = xT   [I, M]  (streamed; stationary operand of the matmuls)
      kxn = wT   [I, OC] (dequantized in SBUF on load, then resident)
      out = y    [M, OC]
    """
    global _cached_nc
    if _cached_nc is not None:
        return _cached_nc

    nc = bacc.Bacc(
        "TRN2", target_bir_lowering=False, debug=False, num_devices=N_CORES
    )
    f16, f32 = mybir.dt.float16, mybir.dt.float32

    xT = nc.dram_tensor("xT", [I, M], f16, kind="ExternalInput").ap()
    wT = nc.dram_tensor("wT", [I, OC], f16, kind="ExternalInput").ap()
    srT = nc.dram_tensor("srT", [I, OC], f16, kind="ExternalInput").ap()
    bias_rep = nc.dram_tensor("bias_rep", [P, OC], f32, kind="ExternalInput").ap()
    y = nc.dram_tensor("y", [M, OC], f16, kind="ExternalOutput").ap()

    with tile.TileContext(nc) as tc:
        from contextlib import ExitStack

        with ExitStack() as ctx:
            kxm_pool = ctx.enter_context(tc.tile_pool(name="kxm_pool", bufs=28))
            kxn_pool = ctx.enter_context(tc.tile_pool(name="kxn_pool", bufs=9))
            sdeq_pool = ctx.enter_context(tc.tile_pool(name="sdeq", bufs=3))
            const_pool = ctx.enter_context(tc.tile_pool(name="const", bufs=1))
            warm_ps = ctx.enter_context(
                tc.tile_pool(name="warm_ps", bufs=1, space="PSUM")
            )

            # HAM pre-warm: dummy matmuls spanning the head-DMA window so the
            # PE clock-gate reaches 8/8 (2.4 GHz) and STAYS there until the
            # first real matmul (~17us in).  Tiles live in the main pools so
            # the allocator doesn't create false space-reuse dependencies
            # that would stall the input DMA stream behind the warmup.
            warm_t = const_pool.tile([P, 512], f16)
            nc.vector.memset(warm_t[:], 0.0)
            warm_p = warm_ps.tile([P, 512], f32)
            for _ in range(34):
                nc.tensor.matmul(
                    warm_p[:], warm_t[:, :P], warm_t[:], start=True, stop=True
                )

            # Weight-side/epilogue DMAs are triggered from the Scalar
            # sequencer (otherwise idle here): DMA descriptor generation
            # (DIRECT2D) costs ~0.7-2us of sequencer time PER dma_start and
            # serializes, so Sync carries only the x stream.
            bias_sb = const_pool.tile([P, OC], f32)
            nc.scalar.dma_start(bias_sb[:], bias_rep[:, :])

            kxm_shape = ShapeInfo(pdims=((P, I // P),), fdims=(M,))
            _, kxn_shape = dma_from_dram_kxn(kxn_pool, wT)

            wT_tiled = wT.rearrange("(po pi) f -> pi po f", pi=P)
            srT_tiled = srT.rearrange("(po pi) f -> pi po f", pi=P)
            xT_tiled = xT.rearrange("(po pi) f -> pi po f", pi=P)
            K_SUB = 4  # k-subtiles per 512-deep k-tile

            s0_dma = []

            def kxn_producer(nc, md):
                # Load the weight tile and the matching slice of the
                # host-replicated scales^T, then dequantize per-subtile so
                # the first matmuls only wait on subtile 0's multiply:
                # w_deq[i, o] = w[i, o] * scales[o, i // G].
                t = kxn_pool.tile([P, md.k_subtiles, md.n_tile], f16, tag="wdeq")
                # k=0's inputs are triggered from the (idle) Scalar/GpSimd
                # sequencers so their descriptor generation runs in parallel
                # with x0's on Sync — the first matmul's critical path.
                # Later k-tiles ride Sync's deep HWDGE ring with the x flow.
                w_eng = nc.scalar if md.k_tile_idx == 0 else nc.sync
                s_eng = nc.gpsimd if md.k_tile_idx == 0 else nc.sync
                w_eng.dma_start(t[:], wT_tiled[:, ts(md.k_tile_idx, md.k_subtiles), :])
                s = sdeq_pool.tile([P, md.k_subtiles, md.n_tile], f16)
                si = s_eng.dma_start(
                    s[:], srT_tiled[:, ts(md.k_tile_idx, md.k_subtiles), :]
                )
                if md.k_tile_idx == 0 and not s0_dma:
                    s0_dma.append(si.ins)
                for ks in range(md.k_subtiles):
                    nc.vector.tensor_mul(t[:, ks, :], t[:, ks, :], s[:, ks, :])
                return t

            def kxm_producer(nc, md):
                t = kxm_pool.tile([P, md.k_subtiles, md.m_tile], f16, tag="kxm")
                di = nc.sync.dma_start(
                    t[:],
                    xT_tiled[
                        :,
                        ts(md.k_tile_idx, md.k_subtiles),
                        ds(md.m_tile_idx * md.m_tile, md.m_tile),
                    ],
                )
                if md.k_tile_idx == 1 and md.m_tile_idx == 0 and s0_dma:
                    # Let k=0's w/scales transfers finish before the x
                    # prefetch flood claims the DMA queues, so the first
                    # matmul's dequant chain completes ASAP.
                    _add_dep_helper(
                        di.ins,
                        s0_dma[0],
                        sync=True,
                        reason="x prefetch yields queues to k0 dequant inputs",
                    )
                return t

            def bias_reducer(nc, psum, sbuf, md):
                # sbuf(fp16) = psum(fp32) + bias(fp32), fused cast on DVE.
                n0 = md.n_tile_idx * md.n_tile + md.n_subtile_idx * md.n_subtile
                nc.vector.tensor_tensor(
                    sbuf,
                    psum,
                    bias_sb[:, ds(n0, md.n_subtile_slice_size)],
                    mybir.AluOpType.add,
                )

            y_tiled = y.rearrange("(po pi) f -> pi po f", pi=P)

            def mxn_consumer(nc, mxn_tile, md):
                nc.scalar.dma_start(
                    y_tiled[
                        :,
                        ts(md.m_tile_idx, md.m_subtiles),
                        ds(md.n_tile_idx * md.n_tile, md.n_tile),
                    ],
                    mxn_tile[:, :, :],
                )

            composable_matmul_tile_kernel(
                tc=tc,
                kxm_shape=kxm_shape,
                kxn_shape=kxn_shape,
                output_type=mybir.dt.float16,
                kxm_producer=kxm_producer,
                kxn_producer=kxn_producer,
                mxn_consumer=mxn_consumer,
                mxn_subtile_reducer=bias_reducer,
                psum_n_bufs=1,
                cache_tiles=True,
            )

    nc.compile()
    _cached_nc = nc
    return nc


def kernel(x, weight, scales, bias, types, g, _want_exec_time=False):
    assert int(g) == G
    x = np.asarray(x)
    weight = np.asarray(weight)
    scales = np.asarray(scales)
    bias = np.asarray(bias)
    assert x.shape == (B, S, I) and weight.shape == (O, I)

    nc = _build_bass()

    # Host-side layout: transposes + per-core shards (no math here).
    xT = np.ascontiguousarray(x.reshape(M, I).T)  # [I, M] fp16
    wT = np.ascontiguousarray(weight.T)  # [I, O] fp16
    # scales^T replicated over each group of G input rows -> [I, O]
    srT = np.ascontiguousarray(np.repeat(scales, G, axis=1).T)
    bias_rep = np.broadcast_to(
        bias.astype(np.float32)[None, :], (P, O)
    )  # [128, O] fp32

    in_maps = []
    for c in range(N_CORES):
        sl = slice(c * OC, (c + 1) * OC)
        in_maps.append(
            {
                "xT": xT,
                "wT": np.ascontiguousarray(wT[:, sl]),
                "srT": np.ascontiguousarray(srT[:, sl]),
                "bias_rep": np.ascontiguousarray(bias_rep[:, sl]),
            }
        )

    res = run_bass_kernel_spmd(
        nc, in_maps, core_ids=list(range(N_CORES)), trace=_want_exec_time
    )

    y = np.empty((M, O), dtype=np.float16)
    for c in range(N_CORES):
        y[:, c * OC : (c + 1) * OC] = res.results[c]["y"]
    out = y.reshape(B, S, O)
    if _want_exec_time:
        return out, res.exec_time_ns
    return out


# revision 39
# speedup vs baseline: 1.0039x; 1.0039x over previous
"""Grouped-dequant GEMM (y = x @ (W * group_scales)^T + bias) on 8 TRN2 NeuronCores.

Tensor-parallel (column) sharding: each core owns O/8 = 512 output features.
x is replicated; weight/scales/bias are sharded along out_features; output
shards are concatenated on the host. All FLOPs (dequant multiply, GEMM, bias
add) run on device; the host only does sharding + layout transforms.

Self-contained: hardcodes shapes from the problem spec.
  x      (4, 2048, 4096) fp16
  weight (4096, 4096)    fp16
  scales (4096, 32)      fp16   group size g=128 along in_features
  bias   (4096,)         fp16
  types  (64, 32)        int32  (unused by the exact-dequant reference math)
"""

import sys
import types as _types

sys.path.insert(0, "/opt/trn_rl_repo")


def _install_ntff_hook_shim():
    """antenv.axon_hooks is missing in this image; register the NTFF profile
    hook from trn_agent_boot so run_bass_kernel_spmd(trace=True) works."""
    if "antenv.axon_hooks" in sys.modules:
        return
    mod = _types.ModuleType("antenv.axon_hooks")
    try:
        import trn_agent_boot.trn_boot as tb

        hook = tb._ntff_profile_via_ctypes("/opt/axon/libaxon_pjrt.so")
    except Exception:
        hook = None
    mod.get_axon_ntff_profile_hook = lambda: hook
    mod.set_axon_ntff_profile_hook = lambda h: None
    sys.modules["antenv.axon_hooks"] = mod


_install_ntff_hook_shim()

import numpy as np

import concourse.bacc as bacc
import concourse.mybir as mybir
import concourse.tile as tile
from concourse.bass import ds, ts
from concourse.bass_utils import run_bass_kernel_spmd
from concourse.bass import _add_dep_helper
from concourse.kernels.tile_matmul import (
    ShapeInfo,
    composable_matmul_tile_kernel,
    dma_from_dram_kxn,
)

B, S, I, O, G = 4, 2048, 4096, 4096, 128
N_CORES = 8
OC = O // N_CORES  # 512 output features per core
M = B * S  # 8192 tokens
P = 128

_cached_nc = None


def _build_bass():
    """Build + compile the per-core Bass program (same graph on all 8 cores).

    Computes y = xT.T @ w_deqT + bias where
      kxm = xT   [I, M]  (streamed; stationary operand of the matmuls)
      kxn = wT   [I, OC] (dequantized in SBUF on load, then resident)
      out = y    [M, OC]
    """
    global _cached_nc
    if _cached_nc is not None:
        return _cached_nc

    nc = bacc.Bacc(
        "TRN2", target_bir_lowering=False, debug=False, num_devices=N_CORES
    )
    f16, f32 = mybir.dt.float16, mybir.dt.float32

    xT = nc.dram_tensor("xT", [I, M], f16, kind="ExternalInput").ap()
    wT = nc.dram_tensor("wT", [I, OC], f16, kind="ExternalInput").ap()
    srT = nc.dram_tensor("srT", [I, OC], f16, kind="ExternalInput").ap()
    bias_rep = nc.dram_tensor("bias_rep", [P, OC], f32, kind="ExternalInput").ap()
    y = nc.dram_tensor("y", [M, OC], f16, kind="ExternalOutput").ap()

    with tile.TileContext(nc) as tc:
        from contextlib import ExitStack

        with ExitStack() as ctx:
            kxm_pool = ctx.enter_context(tc.tile_pool(name="kxm_pool", bufs=28))
            kxn_pool = ctx.enter_context(tc.tile_pool(name="kxn_pool", bufs=9))
            sdeq_pool = ctx.enter_context(tc.tile_pool(name="sdeq", bufs=3))
            const_pool = ctx.enter_context(tc.tile_pool(name="const", bufs=1))

            # Weight-side/epilogue DMAs are triggered from the Scalar
            # sequencer (otherwise idle here): DMA descriptor generation
            # (DIRECT2D) costs ~0.7-2us of sequencer time PER dma_start and
            # serializes, so Sync carries only the x stream.
            bias_sb = const_pool.tile([P, OC], f32)
            nc.scalar.dma_start(bias_sb[:], bias_rep[:, :])

            kxm_shape = ShapeInfo(pdims=((P, I // P),), fdims=(M,))
            _, kxn_shape = dma_from_dram_kxn(kxn_pool, wT)

            wT_tiled = wT.rearrange("(po pi) f -> pi po f", pi=P)
            srT_tiled = srT.rearrange("(po pi) f -> pi po f", pi=P)
            xT_tiled = xT.rearrange("(po pi) f -> pi po f", pi=P)
            K_SUB = 4  # k-subtiles per 512-deep k-tile

            s0_dma = []

            def kxn_producer(nc, md):
                # Load the weight tile and the matching slice of the
                # host-replicated scales^T, then dequantize per-subtile so
                # the first matmuls only wait on subtile 0's multiply:
                # w_deq[i, o] = w[i, o] * scales[o, i // G].
                t = kxn_pool.tile([P, md.k_subtiles, md.n_tile], f16, tag="wdeq")
                # k=0's inputs are triggered from the (idle) Scalar/GpSimd
                # sequencers so their descriptor generation runs in parallel
                # with x0's on Sync — the first matmul's critical path.
                # Later k-tiles ride Sync's deep HWDGE ring with the x flow.
                w_eng = nc.scalar if md.k_tile_idx == 0 else nc.sync
                s_eng = nc.gpsimd if md.k_tile_idx == 0 else nc.sync
                w_eng.dma_start(t[:], wT_tiled[:, ts(md.k_tile_idx, md.k_subtiles), :])
                s = sdeq_pool.tile([P, md.k_subtiles, md.n_tile], f16)
                si = s_eng.dma_start(
                    s[:], srT_tiled[:, ts(md.k_tile_idx, md.k_subtiles), :]
                )
                if md.k_tile_idx == 0 and not s0_dma:
                    s0_dma.append(si.ins)
                for ks in range(md.k_subtiles):
                    nc.vector.tensor_mul(t[:, ks, :], t[:, ks, :], s[:, ks, :])
                return t

            def kxm_producer(nc, md):
                t = kxm_pool.tile([P, md.k_subtiles, md.m_tile], f16, tag="kxm")
                di = nc.sync.dma_start(
                    t[:],
                    xT_tiled[
                        :,
                        ts(md.k_tile_idx, md.k_subtiles),
                        ds(md.m_tile_idx * md.m_tile, md.m_tile),
                    ],
                )
                if md.k_tile_idx == 1 and md.m_tile_idx == 0 and s0_dma:
                    # Let k=0's w/scales transfers finish before the x
                    # prefetch flood claims the DMA queues, so the first
                    # matmul's dequant chain completes ASAP.
                    _add_dep_helper(
                        di.ins,
                        s0_dma[0],
                        sync=True,
                        reason="x prefetch yields queues to k0 dequant inputs",
                    )
                return t

            def bias_reducer(nc, psum, sbuf, md):
                # sbuf(fp16) = psum(fp32) + bias(fp32), fused cast on DVE.
                n0 = md.n_tile_idx * md.n_tile + md.n_subtile_idx * md.n_subtile
                nc.vector.tensor_tensor(
                    sbuf,
                    psum,
                    bias_sb[:, ds(n0, md.n_subtile_slice_size)],
                    mybir.AluOpType.add,
                )

            y_tiled = y.rearrange("(po pi) f -> pi po f", pi=P)

            def mxn_consumer(nc, mxn_tile, md):
                nc.scalar.dma_start(
                    y_tiled[
                        :,
                        ts(md.m_tile_idx, md.m_subtiles),
                        ds(md.n_tile_idx * md.n_tile, md.n_tile),
                    ],
                    mxn_tile[:, :, :],
                )

            composable_matmul_tile_kernel(
                tc=tc,
                kxm_shape=kxm_shape,
                kxn_shape=kxn_shape,
                output_type=mybir.dt.float16,
                kxm_producer=kxm_producer,
                kxn_producer=kxn_producer,
                mxn_consumer=mxn_consumer,
                mxn_subtile_reducer=bias_reducer,
                psum_n_bufs=2,
                cache_tiles=True,
            )

    nc.compile()
    _cached_nc = nc
    return nc


def kernel(x, weight, scales, bias, types, g, _want_exec_time=False):
    assert int(g) == G
    x = np.asarray(x)
    weight = np.asarray(weight)
    scales = np.asarray(scales)
    bias = np.asarray(bias)
    assert x.shape == (B, S, I) and weight.shape == (O, I)

    nc = _build_bass()

    # Host-side layout: transposes + per-core shards (no math here).
    xT = np.ascontiguousarray(x.reshape(M, I).T)  # [I, M] fp16
    wT = np.ascontiguousarray(weight.T)  # [I, O] fp16
    # scales^T replicated over each group of G input rows -> [I, O]
    srT = np.ascontiguousarray(np.repeat(scales, G, axis=1).T)
    bias_rep = np.broadcast_to(
        bias.astype(np.float32)[None, :], (P, O)
    )  # [128, O] fp32

    in_maps = []
    for c in range(N_CORES):
        sl = slice(c * OC, (c + 1) * OC)
        in_maps.append(
            {
                "xT": xT,
                "wT": np.ascontiguousarray(wT[:, sl]),
                "srT": np.ascontiguousarray(srT[:, sl]),
                "bias_rep": np.ascontiguousarray(bias_rep[:, sl]),
            }
        )

    res = run_bass_kernel_spmd(
        nc, in_maps, core_ids=list(range(N_CORES)), trace=_want_exec_time
    )

    y = np.empty((M, O), dtype=np.float16)
    for c in range(N_CORES):
        y[:, c * OC : (c + 1) * OC] = res.results[c]["y"]
    out = y.reshape(B, S, O)
    if _want_exec_time:
        return out, res.exec_time_ns
    return out
